# revision 14
# baseline (speedup 1.0000x reference)
"""DenseGTVConv Trainium2 kernel.

out = (I - (D - A~)) @ (x @ W) + bias,  A~ = adj / clamp(pairwise_L1(xW), 1e-3)

Sharding: 8 cores = batch (2) x row-blocks (4 x 256 rows). Each core gets the
full x of its batch (needed on the j side), its 256-row slice of adj, and
computes its 256-row slice of the output.

Self-contained: hardcoded shapes for B=2, N=1024, F_in=128, F_out=64.
"""
import sys

sys.path.insert(0, "/opt/trn_rl_repo")

from contextlib import ExitStack

import numpy as np

import concourse.bass as bass
import concourse.bacc as bacc
import concourse.tile as tile
from concourse import mybir
from concourse._compat import with_exitstack
from concourse.bass_utils import run_bass_kernel_spmd

F32 = mybir.dt.float32
BF16 = mybir.dt.bfloat16


def _register_absdiff():
    """Custom DVE op: out = |in0 - s0| in one pass (ISA ALU ABSOLUTE_DIFF)."""
    import re

    from concourse import dve_ops as D
    from concourse.dve_spec import Bin, Spec, Src0, C0
    from concourse.dve_uop import AluOp as UAlu

    if "TS_ABS_DIFF" in D._SUB_OPCODE_FOR_NAME:
        return next(o for o in D.OPS if o.name == "TS_ABS_DIFF")
    spec = Spec(
        body=Bin(UAlu.ABSOLUTE_DIFF, Src0, C0),
        reference=lambda in0, in1, s0, s1, imm2: np.abs(
            in0.astype(np.float32) - s0
        ),
    )
    op = D.DveOp("TS_ABS_DIFF", spec, subdim=False, uops_sha={}, perf_en={"v3": True})
    D.OPS.append(op)
    D.CUSTOM_DVE_SPECS["TS_ABS_DIFF"] = spec
    D._SUB_OPCODE_FOR_NAME["TS_ABS_DIFF"] = max(D._SUB_OPCODE_FOR_NAME.values()) + 1
    for ver in ("v3",):
        try:
            op.compile(ver)
        except ValueError as e:
            m = re.search(r'uops_sha\["' + ver + r'"\]="([0-9a-f]+)"', str(e))
            assert m, str(e)
            op.uops_sha[ver] = m.group(1)
    return op


ABSDIFF_OP = _register_absdiff()

B, N, C, F = 2, 1024, 128, 64  # batch, nodes, f_in, f_out
R = 256  # rows per core
NCH = N // 128  # 8 column/row chunks of 128
NPAIR = R // 2  # 128 i-pairs per core
ROUND = 64  # pairs per PSUM round
CLAMP = 1e-3

# Packed setup input, already transposed host-side, laid out [128, 1408]:
#   cols    0:1024 : xT      (x_b.T)
#   cols 1024:1280 : xrT     (x_rows.T)
#   cols 1280:1344 : W       [128, 64]
#   cols 1344:1408 : bias in partition 0, cols 0:64
XALL_COLS = 1408


@with_exitstack
def _body(ctx: ExitStack, tc: "tile.TileContext", io: dict):
    nc = tc.nc
    const = ctx.enter_context(tc.tile_pool(name="const", bufs=1))
    tmp_pool = ctx.enter_context(tc.tile_pool(name="tmp", bufs=8))
    ad_pool = ctx.enter_context(tc.tile_pool(name="ad", bufs=2))
    recip_pool = ctx.enter_context(tc.tile_pool(name="recip", bufs=2))
    mod_pool = ctx.enter_context(tc.tile_pool(name="mod", bufs=2))
    modbf_pool = ctx.enter_context(tc.tile_pool(name="modbf", bufs=2))
    setup_ps = ctx.enter_context(tc.tile_pool(name="sps", bufs=2, space="PSUM"))
    ad_ps = ctx.enter_context(tc.tile_pool(name="adps", bufs=2, space="PSUM"))
    trfin_ps = ctx.enter_context(tc.tile_pool(name="trfin", bufs=2, space="PSUM"))

    # ---- input DMAs ----
    xall = const.tile([128, XALL_COLS], F32)
    nc.sync.dma_start(xall[:], io["xall"][:])
    adjq = []
    for q in range(2):
        a = const.tile([128, N], F32, tag=f"adj{q}", name=f"adj{q}")
        nc.sync.dma_start(a[:], io["adj_rows"][128 * q : 128 * q + 128, :])
        adjq.append(a)

    xT = xall[:, 0:N]
    xrT = xall[:, N : N + R]
    w_sb = xall[:, N + R : N + R + F]
    bias_sb = xall[0:1, N + R + F : N + R + 2 * F]

    # ---- xwT -> dbl (bf16, f stacked twice on partitions) ----
    dbl = const.tile([128, N], F32)
    for h in range(2):
        ps = setup_ps.tile([128, 512], F32, tag="sps", name="sps")
        nc.tensor.matmul(
            ps[0:64, :], w_sb, xT[:, 512 * h : 512 * h + 512], start=True, stop=True
        )
        nc.scalar.copy(dbl[0:64, 512 * h : 512 * h + 512], ps[0:64, :])
    nc.scalar.copy(dbl[64:128, :], dbl[0:64, :])

    # ---- xw (bf16, j on partitions per chunk) for the final matmul rhs ----
    xw_bf = const.tile([128, NCH * F], BF16)
    for c in range(NCH):
        ps = setup_ps.tile([128, 512], F32, tag="sps", name="sps")
        nc.tensor.matmul(
            ps[:, 0:F], xT[:, 128 * c : 128 * c + 128], w_sb, start=True, stop=True
        )
        nc.scalar.copy(xw_bf[:, F * c : F * c + F], ps[:, 0:F])

    # ---- xwT_rows (fp32, exact i-side) -> S scalars; xw_rows for correction ----
    xwT_rows = const.tile([64, R], F32)
    ps = setup_ps.tile([128, 512], F32, tag="sps", name="sps")
    nc.tensor.matmul(ps[0:64, 0:R], w_sb, xrT[:], start=True, stop=True)
    nc.scalar.copy(xwT_rows[:], ps[0:64, 0:R])

    xw_rows = const.tile([128, 2 * F], F32)
    for q in range(2):
        ps = setup_ps.tile([128, 512], F32, tag="sps", name="sps")
        nc.tensor.matmul(
            ps[:, 0:F], xrT[:, 128 * q : 128 * q + 128], w_sb, start=True, stop=True
        )
        nc.scalar.copy(xw_rows[:, F * q : F * q + F], ps[:, 0:F])

    S = const.tile([128, NPAIR], F32)
    nc.vector.tensor_copy(S[0:64, :], xwT_rows[:, 0:R:2])
    nc.vector.tensor_copy(S[64:128, :], xwT_rows[:, 1:R:2])
    negS = const.tile([128, NPAIR], F32)
    nc.vector.tensor_scalar(negS[:], S[:], -1.0, None, mybir.AluOpType.mult)

    # ---- E_big sliding reduction matrix (bf16 0/1) ----
    Eb = const.tile([128, 2 * ROUND + 126], BF16)
    nc.vector.memset(Eb[:], 0.0)
    nc.vector.memset(Eb[0:64, 126:127], 1.0)
    nc.vector.memset(Eb[64:128, 127:128], 1.0)

    # ---- bias broadcast [128, F] via K=1 matmul ----
    ones1 = const.tile([1, 128], F32)
    nc.scalar.activation(
        ones1[:], xall[0:1, 0:128], mybir.ActivationFunctionType.Copy,
        bias=1.0, scale=0.0,
    )
    bias_bc = const.tile([128, F], F32)
    ps = setup_ps.tile([128, 512], F32, tag="sps", name="sps")
    nc.tensor.matmul(ps[:, 0:F], ones1[:], bias_sb, start=True, stop=True)
    nc.scalar.copy(bias_bc[:], ps[:, 0:F])

    deg = const.tile([128, 2], F32)
    modT = [const.tile([128, R], BF16, tag=f"modT{jc}", name=f"modT{jc}") for jc in range(NCH)]
    out_sb = [const.tile([128, F], F32, tag=f"osb{q}", name=f"osb{q}") for q in range(2)]

    # ---- hot loop over i-pairs ----
    for q in range(2):
        adps = [ad_ps.tile([128, 512], F32, tag=f"adps{k}", name=f"adps{q}_{k}") for k in range(2)]
        for r in range(ROUND):
            t = ROUND * q + r
            tmp = tmp_pool.tile([128, N], BF16, tag="tmp", name="tmp")
            if t % 2 == 1:
                nc.scalar.activation(
                    tmp[:],
                    dbl[:],
                    mybir.ActivationFunctionType.Abs,
                    bias=negS[:, t : t + 1],
                    scale=1.0,
                )
            else:
                nc.vector._custom_dve(
                    ABSDIFF_OP, out=tmp[:], in0=dbl[:], s0=S[:, t : t + 1]
                )
            esl = Eb[:, 126 - 2 * r : 254 - 2 * r]
            for k in range(2):
                nc.tensor.matmul(
                    adps[k][:],
                    esl,
                    tmp[:, 512 * k : 512 * k + 512],
                    start=(r == 0),
                    stop=(r == ROUND - 1),
                )

        # ---- per-round epilogue: clamp-evac, recip, modulate, deg, transpose ----
        ad = ad_pool.tile([128, N], F32, tag="ad", name="ad")
        for k in range(2):
            nc.vector.tensor_scalar(
                ad[:, 512 * k : 512 * k + 512],
                adps[k][:],
                CLAMP,
                None,
                mybir.AluOpType.max,
            )
        if "dbg_ad" in io:
            nc.sync.dma_start(io["dbg_ad"][128 * q : 128 * q + 128, :], ad[:])
        recip = recip_pool.tile([128, N], F32, tag="recip", name="recip")
        nc.vector.reciprocal_approx_fast(recip[:], ad[:])
        mod = mod_pool.tile([128, N], F32, tag="mod", name="mod")
        nc.gpsimd.tensor_tensor(mod[:], adjq[q][:], recip[:], mybir.AluOpType.mult)
        modbf = modbf_pool.tile([128, N], BF16, tag="modbf", name="modbf")
        nc.scalar.activation(
            modbf[:],
            mod[:],
            mybir.ActivationFunctionType.Copy,
            bias=0.0,
            scale=1.0,
            accum_out=deg[:, q : q + 1],
        )
        if "dbg_mod" in io:
            nc.sync.dma_start(io["dbg_mod"][128 * q : 128 * q + 128, :], mod[:])
        for jc in range(NCH):
            nc.sync.dma_start(
                modT[jc][:, 128 * q : 128 * q + 128],
                modbf[:, 128 * jc : 128 * jc + 128],
                transpose=True,
            )

    if "dbg_modT" in io:
        mtf = const.tile([128, R], F32, tag="mtf", name="mtf")
        nc.vector.tensor_copy(mtf[:], modT[0][:])
        nc.sync.dma_start(io["dbg_modT"][:], mtf[:])

    # ---- final: out rows = (1 - deg) * xw_rows + modT.T @ xw + bias ----
    for q in range(2):
        fin = trfin_ps.tile([128, 512], F32, tag="trfin", name=f"fin{q}")
        for jc in range(NCH):
            nc.tensor.matmul(
                fin[:, 0:F],
                modT[jc][:, 128 * q : 128 * q + 128],
                xw_bf[:, F * jc : F * jc + F],
                start=(jc == 0),
                stop=(jc == NCH - 1),
            )
        if "dbg_fin" in io:
            fin_sb = const.tile([128, F], F32, tag=f"dbgfin{q}", name=f"dbgfin{q}")
            nc.vector.tensor_copy(fin_sb[:], fin[:, 0:F])
            nc.sync.dma_start(io["dbg_fin"][128 * q : 128 * q + 128, :], fin_sb[:])
        onemdeg = const.tile([128, 1], F32, tag=f"od{q}", name=f"od{q}")
        nc.vector.tensor_scalar(
            onemdeg[:],
            deg[:, q : q + 1],
            -1.0,
            1.0,
            mybir.AluOpType.mult,
            mybir.AluOpType.add,
        )
        corr = const.tile([128, F], F32, tag=f"corr{q}", name=f"corr{q}")
        nc.vector.tensor_scalar(
            corr[:],
            xw_rows[:, F * q : F * q + F],
            onemdeg[:],
            None,
            mybir.AluOpType.mult,
        )
        nc.vector.tensor_tensor(corr[:], corr[:], bias_bc[:], mybir.AluOpType.add)
        nc.vector.tensor_tensor(out_sb[q][:], corr[:], fin[:, 0:F], mybir.AluOpType.add)
        if "dbg_deg" in io:
            nc.sync.dma_start(io["dbg_deg"][:, q : q + 1], deg[:, q : q + 1])
        nc.sync.dma_start(io["out_block"][128 * q : 128 * q + 128, :], out_sb[q][:])


_CACHE = {}


def _build(debug=False):
    key = ("nc", debug)
    if key in _CACHE:
        return _CACHE[key]
    nc = bacc.Bacc()
    io = {
        "xall": nc.declare_dram_parameter("xall", [C, XALL_COLS], F32, isOutput=False),
        "adj_rows": nc.declare_dram_parameter("adj_rows", [R, N], F32, isOutput=False),
        "out_block": nc.declare_dram_parameter("out_block", [R, F], F32, isOutput=True),
    }
    if debug:
        io["dbg_ad"] = nc.declare_dram_parameter("dbg_ad", [R, N], F32, isOutput=True)
        io["dbg_mod"] = nc.declare_dram_parameter("dbg_mod", [R, N], F32, isOutput=True)
        io["dbg_deg"] = nc.declare_dram_parameter("dbg_deg", [128, 2], F32, isOutput=True)
        io["dbg_fin"] = nc.declare_dram_parameter("dbg_fin", [R, F], F32, isOutput=True)
        io["dbg_modT"] = nc.declare_dram_parameter("dbg_modT", [128, R], F32, isOutput=True)
    with tile.TileContext(nc) as tc:
        _body(tc, io)
    nc.finalize()
    _CACHE[key] = nc
    return nc


def _make_in_maps(x, adj, weight, bias):
    in_maps = []
    for core in range(8):
        b, blk = core // 4, core % 4
        r0 = blk * R
        xall = np.zeros((C, XALL_COLS), dtype=np.float32)
        xall[:, 0:N] = x[b].T
        xall[:, N : N + R] = x[b, r0 : r0 + R].T
        xall[:, N + R : N + R + F] = weight
        xall[0, N + R + F : N + R + 2 * F] = bias
        adj_rows = np.ascontiguousarray(adj[b, r0 : r0 + R]).copy()
        # Zero the self-edge: diag(mod_adj) cancels analytically in
        # out = (I - D + A~) xw, so drop it to avoid the 1000x clamp terms.
        adj_rows[np.arange(R), r0 + np.arange(R)] = 0.0
        in_maps.append({"xall": xall, "adj_rows": adj_rows})
    return in_maps


def run(x, adj, weight, bias, trace=False):
    nc = _build()
    res = run_bass_kernel_spmd(
        nc, _make_in_maps(x, adj, weight, bias), list(range(8)), trace=trace
    )
    out = np.empty((B, N, F), dtype=np.float32)
    for core in range(8):
        b, blk = core // 4, core % 4
        out[b, blk * R : blk * R + R] = res.results[core]["out_block"]
    return out, res


def kernel(x, adj, weight, bias):
    x = np.asarray(x, dtype=np.float32)
    adj = np.asarray(adj, dtype=np.float32)
    weight = np.asarray(weight, dtype=np.float32)
    bias = np.asarray(bias, dtype=np.float32)
    out, _ = run(x, adj, weight, bias, trace=False)
    return out


# revision 15
# speedup vs baseline: 1.1363x; 1.1363x over previous
"""DenseGTVConv Trainium2 kernel.

out = (I - (D - A~)) @ (x @ W) + bias,  A~ = adj / clamp(pairwise_L1(xW), 1e-3)

Sharding: 8 cores = batch (2) x row-blocks (4 x 256 rows). Each core gets the
full x of its batch (needed on the j side), its 256-row slice of adj, and
computes its 256-row slice of the output.

Self-contained: hardcoded shapes for B=2, N=1024, F_in=128, F_out=64.
"""
import sys

sys.path.insert(0, "/opt/trn_rl_repo")

from contextlib import ExitStack

import numpy as np

import concourse.bass as bass
import concourse.bacc as bacc
import concourse.tile as tile
from concourse.masks import make_identity
from concourse import mybir
from concourse._compat import with_exitstack
from concourse.bass_utils import run_bass_kernel_spmd

F32 = mybir.dt.float32
BF16 = mybir.dt.bfloat16


def _register_absdiff():
    """Custom DVE op: out = |in0 - s0| in one pass (ISA ALU ABSOLUTE_DIFF)."""
    import re

    from concourse import dve_ops as D
    from concourse.dve_spec import Bin, Spec, Src0, C0
    from concourse.dve_uop import AluOp as UAlu

    if "TS_ABS_DIFF" in D._SUB_OPCODE_FOR_NAME:
        return next(o for o in D.OPS if o.name == "TS_ABS_DIFF")
    spec = Spec(
        body=Bin(UAlu.ABSOLUTE_DIFF, Src0, C0),
        reference=lambda in0, in1, s0, s1, imm2: np.abs(
            in0.astype(np.float32) - s0
        ),
    )
    op = D.DveOp("TS_ABS_DIFF", spec, subdim=False, uops_sha={}, perf_en={"v3": True})
    D.OPS.append(op)
    D.CUSTOM_DVE_SPECS["TS_ABS_DIFF"] = spec
    D._SUB_OPCODE_FOR_NAME["TS_ABS_DIFF"] = max(D._SUB_OPCODE_FOR_NAME.values()) + 1
    for ver in ("v3",):
        try:
            op.compile(ver)
        except ValueError as e:
            m = re.search(r'uops_sha\["' + ver + r'"\]="([0-9a-f]+)"', str(e))
            assert m, str(e)
            op.uops_sha[ver] = m.group(1)
    return op


ABSDIFF_OP = _register_absdiff()

B, N, C, F = 2, 1024, 128, 64  # batch, nodes, f_in, f_out
R = 256  # rows per core
NCH = N // 128  # 8 column/row chunks of 128
NPAIR = R // 2  # 128 i-pairs per core
ROUND = 64  # pairs per PSUM round
CLAMP = 1e-3

# Packed setup input, already transposed host-side, laid out [128, 1408]:
#   cols    0:1024 : xT      (x_b.T)
#   cols 1024:1280 : xrT     (x_rows.T)
#   cols 1280:1344 : W       [128, 64]
#   cols 1344:1408 : bias in partition 0, cols 0:64
XALL_COLS = 1408


@with_exitstack
def _body(ctx: ExitStack, tc: "tile.TileContext", io: dict):
    nc = tc.nc
    const = ctx.enter_context(tc.tile_pool(name="const", bufs=1))
    tmp_pool = ctx.enter_context(tc.tile_pool(name="tmp", bufs=8))
    ad_pool = ctx.enter_context(tc.tile_pool(name="ad", bufs=2))
    recip_pool = ctx.enter_context(tc.tile_pool(name="recip", bufs=2))
    mod_pool = ctx.enter_context(tc.tile_pool(name="mod", bufs=2))
    modbf_pool = ctx.enter_context(tc.tile_pool(name="modbf", bufs=2))
    setup_ps = ctx.enter_context(tc.tile_pool(name="sps", bufs=2, space="PSUM"))
    ad_ps = ctx.enter_context(tc.tile_pool(name="adps", bufs=2, space="PSUM"))
    trfin_ps = ctx.enter_context(tc.tile_pool(name="trfin", bufs=2, space="PSUM"))

    # ---- input DMAs ----
    xall = const.tile([128, XALL_COLS], F32)
    nc.sync.dma_start(xall[:], io["xall"][:])
    adjq = []
    for q in range(2):
        a = const.tile([128, N], F32, tag=f"adj{q}", name=f"adj{q}")
        nc.sync.dma_start(a[:], io["adj_rows"][128 * q : 128 * q + 128, :])
        adjq.append(a)

    xT = xall[:, 0:N]
    xrT = xall[:, N : N + R]
    w_sb = xall[:, N + R : N + R + F]
    bias_sb = xall[0:1, N + R + F : N + R + 2 * F]

    identb = const.tile([128, 128], BF16)
    make_identity(nc, identb[:])

    # ---- xwT -> dbl (bf16, f stacked twice on partitions) ----
    dbl = const.tile([128, N], F32)
    for h in range(2):
        ps = setup_ps.tile([128, 512], F32, tag="sps", name="sps")
        nc.tensor.matmul(
            ps[0:64, :], w_sb, xT[:, 512 * h : 512 * h + 512], start=True, stop=True
        )
        nc.scalar.copy(dbl[0:64, 512 * h : 512 * h + 512], ps[0:64, :])
    nc.scalar.copy(dbl[64:128, :], dbl[0:64, :])

    # ---- xw (bf16, j on partitions per chunk) for the final matmul rhs ----
    xw_bf = const.tile([128, NCH * F], BF16)
    for c in range(NCH):
        ps = setup_ps.tile([128, 512], F32, tag="sps", name="sps")
        nc.tensor.matmul(
            ps[:, 0:F], xT[:, 128 * c : 128 * c + 128], w_sb, start=True, stop=True
        )
        nc.scalar.copy(xw_bf[:, F * c : F * c + F], ps[:, 0:F])

    # ---- xwT_rows (fp32, exact i-side) -> S scalars; xw_rows for correction ----
    xwT_rows = const.tile([64, R], F32)
    ps = setup_ps.tile([128, 512], F32, tag="sps", name="sps")
    nc.tensor.matmul(ps[0:64, 0:R], w_sb, xrT[:], start=True, stop=True)
    nc.scalar.copy(xwT_rows[:], ps[0:64, 0:R])

    xw_rows = const.tile([128, 2 * F], F32)
    for q in range(2):
        ps = setup_ps.tile([128, 512], F32, tag="sps", name="sps")
        nc.tensor.matmul(
            ps[:, 0:F], xrT[:, 128 * q : 128 * q + 128], w_sb, start=True, stop=True
        )
        nc.scalar.copy(xw_rows[:, F * q : F * q + F], ps[:, 0:F])

    S = const.tile([128, NPAIR], F32)
    nc.vector.tensor_copy(S[0:64, :], xwT_rows[:, 0:R:2])
    nc.vector.tensor_copy(S[64:128, :], xwT_rows[:, 1:R:2])
    negS = const.tile([128, NPAIR], F32)
    nc.vector.tensor_scalar(negS[:], S[:], -1.0, None, mybir.AluOpType.mult)

    # ---- E_big sliding reduction matrix (bf16 0/1) ----
    Eb = const.tile([128, 2 * ROUND + 126], BF16)
    nc.vector.memset(Eb[:], 0.0)
    nc.vector.memset(Eb[0:64, 126:127], 1.0)
    nc.vector.memset(Eb[64:128, 127:128], 1.0)

    # ---- bias broadcast [128, F] via K=1 matmul ----
    ones1 = const.tile([1, 128], F32)
    nc.scalar.activation(
        ones1[:], xall[0:1, 0:128], mybir.ActivationFunctionType.Copy,
        bias=1.0, scale=0.0,
    )
    bias_bc = const.tile([128, F], F32)
    ps = setup_ps.tile([128, 512], F32, tag="sps", name="sps")
    nc.tensor.matmul(ps[:, 0:F], ones1[:], bias_sb, start=True, stop=True)
    nc.scalar.copy(bias_bc[:], ps[:, 0:F])

    deg = const.tile([128, 2], F32)
    modT = [const.tile([128, R], BF16, tag=f"modT{jc}", name=f"modT{jc}") for jc in range(NCH)]
    out_sb = [const.tile([128, F], F32, tag=f"osb{q}", name=f"osb{q}") for q in range(2)]

    # ---- hot loop over i-pairs ----
    for q in range(2):
        adps = [ad_ps.tile([128, 512], F32, tag=f"adps{k}", name=f"adps{q}_{k}") for k in range(2)]
        for r in range(ROUND):
            t = ROUND * q + r
            tmp = tmp_pool.tile([128, N], BF16, tag="tmp", name="tmp")
            if t % 2 == 1:
                nc.scalar.activation(
                    tmp[:],
                    dbl[:],
                    mybir.ActivationFunctionType.Abs,
                    bias=negS[:, t : t + 1],
                    scale=1.0,
                )
            else:
                nc.vector._custom_dve(
                    ABSDIFF_OP, out=tmp[:], in0=dbl[:], s0=S[:, t : t + 1]
                )
            esl = Eb[:, 126 - 2 * r : 254 - 2 * r]
            for k in range(2):
                nc.tensor.matmul(
                    adps[k][:],
                    esl,
                    tmp[:, 512 * k : 512 * k + 512],
                    start=(r == 0),
                    stop=(r == ROUND - 1),
                )

        # ---- per-round epilogue: clamp-evac, recip, modulate, deg, transpose ----
        ad = ad_pool.tile([128, N], F32, tag="ad", name="ad")
        for k in range(2):
            nc.vector.tensor_scalar(
                ad[:, 512 * k : 512 * k + 512],
                adps[k][:],
                CLAMP,
                None,
                mybir.AluOpType.max,
            )
        if "dbg_ad" in io:
            nc.sync.dma_start(io["dbg_ad"][128 * q : 128 * q + 128, :], ad[:])
        recip = recip_pool.tile([128, N], F32, tag="recip", name="recip")
        nc.vector.reciprocal_approx_fast(recip[:], ad[:])
        mod = mod_pool.tile([128, N], F32, tag="mod", name="mod")
        nc.gpsimd.tensor_tensor(mod[:], adjq[q][:], recip[:], mybir.AluOpType.mult)
        modbf = modbf_pool.tile([128, N], BF16, tag="modbf", name="modbf")
        nc.scalar.activation(
            modbf[:],
            mod[:],
            mybir.ActivationFunctionType.Copy,
            bias=0.0,
            scale=1.0,
            accum_out=deg[:, q : q + 1],
        )
        if "dbg_mod" in io:
            nc.sync.dma_start(io["dbg_mod"][128 * q : 128 * q + 128, :], mod[:])
        for jc in range(NCH):
            tr = trfin_ps.tile([128, 128], BF16, tag="trfin", name="tr")
            nc.tensor.transpose(tr[:], modbf[:, 128 * jc : 128 * jc + 128], identb[:])
            nc.scalar.copy(modT[jc][:, 128 * q : 128 * q + 128], tr[:])

    if "dbg_modT" in io:
        mtf = const.tile([128, R], F32, tag="mtf", name="mtf")
        nc.vector.tensor_copy(mtf[:], modT[0][:])
        nc.sync.dma_start(io["dbg_modT"][:], mtf[:])

    # ---- final: out rows = (1 - deg) * xw_rows + modT.T @ xw + bias ----
    for q in range(2):
        fin = trfin_ps.tile([128, 512], F32, tag="trfin", name=f"fin{q}")
        for jc in range(NCH):
            nc.tensor.matmul(
                fin[:, 0:F],
                modT[jc][:, 128 * q : 128 * q + 128],
                xw_bf[:, F * jc : F * jc + F],
                start=(jc == 0),
                stop=(jc == NCH - 1),
            )
        if "dbg_fin" in io:
            fin_sb = const.tile([128, F], F32, tag=f"dbgfin{q}", name=f"dbgfin{q}")
            nc.vector.tensor_copy(fin_sb[:], fin[:, 0:F])
            nc.sync.dma_start(io["dbg_fin"][128 * q : 128 * q + 128, :], fin_sb[:])
        onemdeg = const.tile([128, 1], F32, tag=f"od{q}", name=f"od{q}")
        nc.vector.tensor_scalar(
            onemdeg[:],
            deg[:, q : q + 1],
            -1.0,
            1.0,
            mybir.AluOpType.mult,
            mybir.AluOpType.add,
        )
        corr = const.tile([128, F], F32, tag=f"corr{q}", name=f"corr{q}")
        nc.vector.tensor_scalar(
            corr[:],
            xw_rows[:, F * q : F * q + F],
            onemdeg[:],
            None,
            mybir.AluOpType.mult,
        )
        nc.vector.tensor_tensor(corr[:], corr[:], bias_bc[:], mybir.AluOpType.add)
        nc.vector.tensor_tensor(out_sb[q][:], corr[:], fin[:, 0:F], mybir.AluOpType.add)
        if "dbg_deg" in io:
            nc.sync.dma_start(io["dbg_deg"][:, q : q + 1], deg[:, q : q + 1])
        nc.sync.dma_start(io["out_block"][128 * q : 128 * q + 128, :], out_sb[q][:])


_CACHE = {}


def _build(debug=False):
    key = ("nc", debug)
    if key in _CACHE:
        return _CACHE[key]
    nc = bacc.Bacc()
    io = {
        "xall": nc.declare_dram_parameter("xall", [C, XALL_COLS], F32, isOutput=False),
        "adj_rows": nc.declare_dram_parameter("adj_rows", [R, N], F32, isOutput=False),
        "out_block": nc.declare_dram_parameter("out_block", [R, F], F32, isOutput=True),
    }
    if debug:
        io["dbg_ad"] = nc.declare_dram_parameter("dbg_ad", [R, N], F32, isOutput=True)
        io["dbg_mod"] = nc.declare_dram_parameter("dbg_mod", [R, N], F32, isOutput=True)
        io["dbg_deg"] = nc.declare_dram_parameter("dbg_deg", [128, 2], F32, isOutput=True)
        io["dbg_fin"] = nc.declare_dram_parameter("dbg_fin", [R, F], F32, isOutput=True)
        io["dbg_modT"] = nc.declare_dram_parameter("dbg_modT", [128, R], F32, isOutput=True)
    with tile.TileContext(nc) as tc:
        _body(tc, io)
    nc.finalize()
    _CACHE[key] = nc
    return nc


def _make_in_maps(x, adj, weight, bias):
    in_maps = []
    for core in range(8):
        b, blk = core // 4, core % 4
        r0 = blk * R
        xall = np.zeros((C, XALL_COLS), dtype=np.float32)
        xall[:, 0:N] = x[b].T
        xall[:, N : N + R] = x[b, r0 : r0 + R].T
        xall[:, N + R : N + R + F] = weight
        xall[0, N + R + F : N + R + 2 * F] = bias
        adj_rows = np.ascontiguousarray(adj[b, r0 : r0 + R]).copy()
        # Zero the self-edge: diag(mod_adj) cancels analytically in
        # out = (I - D + A~) xw, so drop it to avoid the 1000x clamp terms.
        adj_rows[np.arange(R), r0 + np.arange(R)] = 0.0
        in_maps.append({"xall": xall, "adj_rows": adj_rows})
    return in_maps


def run(x, adj, weight, bias, trace=False):
    nc = _build()
    res = run_bass_kernel_spmd(
        nc, _make_in_maps(x, adj, weight, bias), list(range(8)), trace=trace
    )
    out = np.empty((B, N, F), dtype=np.float32)
    for core in range(8):
        b, blk = core // 4, core % 4
        out[b, blk * R : blk * R + R] = res.results[core]["out_block"]
    return out, res


def kernel(x, adj, weight, bias):
    x = np.asarray(x, dtype=np.float32)
    adj = np.asarray(adj, dtype=np.float32)
    weight = np.asarray(weight, dtype=np.float32)
    bias = np.asarray(bias, dtype=np.float32)
    out, _ = run(x, adj, weight, bias, trace=False)
    return out
